# revision 12
# baseline (speedup 1.0000x reference)
"""Trainium2 Bass kernel for nn_DependencyParser (BiLSTM + biaffine-style
pairwise scorer).

Strategy (8 NeuronCores, no collectives):
  - Every core runs the identical 2-layer BiLSTM over the 769-token
    sequence (the recurrence is inherently sequential; replication is
    cheaper than any per-step communication).
  - The pairwise score computation scores[i, j] = w2 . tanh(A_i + B_j + b1)
    is sharded over the head axis i: core c computes heads
    [96*c, 96*c + 97) (97 heads each, one overlapping row; 769 total).
  - Scores are produced transposed (j on partitions, heads on free dim) so
    PSUM evacuation is cheap; the host reassembles/transposes, adds b2 and
    sets the diagonal to -inf.

Cell math (all-tanh formulation; sigma(x) = 0.5*tanh(x/2) + 0.5):
  - Weights for gates i, f, o are pre-scaled by 0.5 host-side, so ONE
    Tanh activation per step covers all 4 gates.
  - The kernel tracks h2 = 2h and chat = 2c:
      u     = (Ti + 1) * Tg                    (STT)
      chat  = (Tf * hc + hc) + u, hc = chat/2  (affine_then_add custom op, TS)
      Tc    = tanh(0.5 * chat)                 (ACT, = tanh(c))
      h2    = (To + 1) * Tc                    (STT, = 2h)
    The factors of 2 are folded into Whh / layer-2 Wih / W1 host-side.
"""

import os
import numpy as np

import concourse.bass as bass
import concourse.mybir as mybir
import concourse.tile as tile
from concourse import bacc
from concourse.bass import ds
from concourse.bass_utils import run_bass_kernel_spmd

F32 = mybir.dt.float32
AF = mybir.ActivationFunctionType
ALU = mybir.AluOpType

T = 769          # sequence length incl. root
N = 768          # dependents
H = 128          # lstm hidden per direction
NL = 2           # layers
F = 128          # fc hidden
NH = 97          # heads per core
HB = 96          # head block stride (base = pid*96)
NCORES = 8

# cpack free-dim layout (one (128, CPK) f32 DRAM tensor, single region map)
OFF_XT = 0                     # x.T in 2 chunks of (128, T)
OFF_WIH = OFF_XT + 2 * T       # 4 passes x 2 kchunks x 512
OFF_WHH = OFF_WIH + 4 * 2 * 512
OFF_W1 = OFF_WHH + 4 * 512     # 2 kchunks x 256 ([W1a.T | W1b.T] cols)
OFF_B1 = OFF_W1 + 2 * 256      # (128, 1) b1
OFF_W2 = OFF_B1 + 1            # (128, 1) w2
CPK = OFF_W2 + 1

_CACHE = {}


def pair(ap_a, ap_b):
    """One AP covering two (128, 1)-ish columns of the same tensor:
    result reads/writes [a, b] as a (128, 2) operand."""
    assert ap_a.tensor is ap_b.tensor
    d = ap_b.offset - ap_a.offset
    assert d > 0, "pair() needs ap_a before ap_b"
    return bass.AP(
        tensor=ap_a.tensor,
        offset=ap_a.offset,
        ap=[list(ap_a.ap[0]), [d, 2]],
    )


def build_program():
    nc = bacc.Bacc(None, target_bir_lowering=False, num_devices=NCORES)

    cpack = nc.dram_tensor("cpack", (128, CPK), F32, kind="ExternalInput")
    # brow: single partition row: 4 pass-bias blocks of 512, then T ones
    brow = nc.dram_tensor("brow", (1, 4 * 512 + T), F32, kind="ExternalInput")
    ascr = nc.dram_tensor("ascr", (128, T), F32, kind="Internal")
    st_out = nc.dram_tensor("st_out", (6, 128, NH), F32, kind="ExternalOutput")

    with tile.TileContext(nc) as tc:
        with (
            tc.tile_pool(name="const", bufs=1) as const,
            tc.tile_pool(name="state", bufs=1) as state,
        ):
            cp = const.tile([128, CPK], F32)
            # split the constant load across DMA queues for bandwidth
            bounds = [
                0, T, OFF_WIH,
                OFF_WIH + 1024, OFF_WIH + 2048, OFF_WIH + 3072,
                OFF_WHH, OFF_WHH + 1024, OFF_W1, CPK,
            ]
            for a, b in zip(bounds[:-1], bounds[1:]):
                if b > a:
                    nc.sync.dma_start(out=cp[:, a:b], in_=cpack[:, a:b])
            br = const.tile([1, 4 * 512 + T], F32)
            nc.sync.dma_start(out=br[:, :], in_=brow[:, :])

            def xt(kc):
                return cp[:, OFF_XT + kc * T : OFF_XT + (kc + 1) * T]

            def wih(p, kc):  # (128, 512) lhsT slab for pass p, K-chunk kc
                o = OFF_WIH + (p * 2 + kc) * 512
                return cp[:, o : o + 512]

            def whh(p, g):  # (128, 128) lhsT for pass p, gate g
                o = OFF_WHH + p * 512 + g * 128
                return cp[:, o : o + 128]

            def bias_row(p):  # (1, 512)
                return br[0:1, p * 512 : (p + 1) * 512]

            ones_row = br[0:1, 2048 : 2048 + T]

            # persistent LSTM state/scratch tiles
            Hl = [
                state.tile([128, 2, T], F32, tag=f"H{l}", name=f"H{l}")
                for l in range(NL)
            ]
            Tt = state.tile([128, 2, 4], F32)   # tanh gates [i,f,o,g] per stream
            u_t = state.tile([128, 2], F32)
            ct = state.tile([128, 2], F32)      # chat
            hc = state.tile([128, 2], F32)      # chat/2
            Tc = state.tile([128, 2], F32)      # tanh(c)

            for layer in range(NL):
                nc.vector.memset(hc[:, :], 0.0)  # c0 = 0 per layer
                pf, pb = 2 * layer, 2 * layer + 1
                if layer == 0:
                    X = [xt(0), xt(1)]
                else:
                    X = [Hl[layer - 1][:, 0, :], Hl[layer - 1][:, 1, :]]
                Hcur = Hl[layer]

                # fwd chunks cover t in [cb, cb+C); bwd chunk A covers
                # t in [257, 769) (natural order, walked right-to-left),
                # chunk B covers t in [0, 257).
                with tc.tile_pool(name=f"ps{layer}", bufs=1, space="PSUM") as psp:
                    gf = gb = None
                    fwd_cb = bwd_lo = None

                    def proj(gtile, p, lo, width):
                        for g in range(4):
                            for kc in range(2):
                                nc.tensor.matmul(
                                    gtile[:, g, 0:width],
                                    wih(p, kc)[:, g * 128 : (g + 1) * 128],
                                    X[kc][:, lo : lo + width],
                                    start=(kc == 0),
                                    stop=False,
                                    skip_group_check=True,
                                )
                            nc.tensor.matmul(
                                gtile[:, g, 0:width],
                                bias_row(p)[:, g * 128 : (g + 1) * 128],
                                ones_row[:, lo : lo + width],
                                start=False,
                                stop=True,
                                skip_group_check=True,
                            )

                    for k in range(T):
                        if k == 0 or k == 512:
                            # (re)fill both streams' gate psum tiles
                            gf = psp.tile([128, 4, 512], F32, tag="gf")
                            gb = psp.tile([128, 4, 512], F32, tag="gb")
                            if k == 0:
                                fwd_cb, fwd_w = 0, 512
                                bwd_lo, bwd_w = 257, 512
                            else:
                                fwd_cb, fwd_w = 512, 257
                                bwd_lo, bwd_w = 0, 257
                            proj(gf, pf, fwd_cb, fwd_w)
                            proj(gb, pb, bwd_lo, bwd_w)

                        tf = k            # fwd step position
                        tb = T - 1 - k    # bwd step position
                        cf = tf - fwd_cb
                        cb = tb - bwd_lo

                        if k > 0:
                            for g in range(4):
                                nc.tensor.matmul(
                                    gf[:, g, cf : cf + 1],
                                    whh(pf, g),
                                    Hcur[:, 0, tf - 1 : tf],
                                    start=False,
                                    stop=True,
                                    skip_group_check=True,
                                )
                            for g in range(4):
                                nc.tensor.matmul(
                                    gb[:, g, cb : cb + 1],
                                    whh(pb, g),
                                    Hcur[:, 1, tb + 1 : tb + 2],
                                    start=False,
                                    stop=True,
                                    skip_group_check=True,
                                )

                        # gates -> tanh (one ACT per stream; PSUM source)
                        nc.scalar.activation(Tt[:, 0, :], gf[:, :, cf], AF.Tanh)
                        nc.scalar.activation(Tt[:, 1, :], gb[:, :, cb], AF.Tanh)

                        # u = (Ti + 1) * Tg        (both streams at once)
                        nc.vector.scalar_tensor_tensor(
                            u_t[:, :], Tt[:, :, 0], 1.0, Tt[:, :, 3],
                            op0=ALU.add, op1=ALU.mult,
                        )
                        # chat = (Tf * hc + hc) + u   (per stream: hc is [P,1])
                        for s in range(2):
                            nc.vector.affine_then_add(
                                ct[:, s : s + 1], Tt[:, s, 1:2], u_t[:, s : s + 1],
                                hc[:, s : s + 1], hc[:, s : s + 1],
                            )
                        nc.vector.tensor_scalar(hc[:, :], ct[:, :], 0.5, None, op0=ALU.mult)
                        # Tc = tanh(chat/2) = tanh(c)
                        nc.scalar.activation(Tc[:, :], ct[:, :], AF.Tanh, scale=0.5)
                        # h2 = (To + 1) * Tc  -> H columns (both streams, pair AP)
                        hpair = pair(Hcur[:, 0, tf : tf + 1], Hcur[:, 1, tb : tb + 1])
                        nc.vector.scalar_tensor_tensor(
                            hpair, Tt[:, :, 2], 1.0, Tc[:, :],
                            op0=ALU.add, op1=ALU.mult,
                        )

            if os.environ.get("KERNEL_DEBUG_DUMP"):
                for l in range(NL):
                    dbg = nc.dram_tensor(f"dbg_h{l}", (128, 2, T), F32, kind="ExternalOutput")
                    nc.sync.dma_start(out=dbg[:, :, :], in_=Hl[l][:, :, :])

            # ---------------- scores phase ----------------
            L2 = Hl[NL - 1]

            def w1(kc, half):  # (128, 128) lhsT: half 0 = W1a, 1 = W1b
                o = OFF_W1 + kc * 256 + half * 128
                return cp[:, o : o + 128]

            with tc.tile_pool(name="pa", bufs=1, space="PSUM") as pap:
                pa = pap.tile([128, T], F32)
                for kc in range(2):
                    for lo, wd in ((0, 512), (512, 257)):
                        nc.tensor.matmul(
                            pa[:, lo : lo + wd],
                            w1(kc, 0),
                            L2[:, kc, lo : lo + wd],
                            start=(kc == 0),
                            stop=(kc == 1),
                            skip_group_check=True,
                        )
                astar = const.tile([128, T], F32)
                nc.scalar.activation(
                    astar[:, :], pa[:, :], AF.Identity,
                    bias=cp[:, OFF_B1 : OFF_B1 + 1], scale=1.0,
                )
            # head-block gather via DRAM roundtrip (dynamic offset DMA)
            nc.sync.dma_start(out=ascr[:, :], in_=astar[:, :])
            ab = const.tile([128, NH], F32)
            base96 = nc.sync.partition_id() * HB
            nc.sync.dma_start(out=ab[:, :], in_=ascr[:, ds(base96, NH)])

            with (
                tc.tile_pool(name="pb", bufs=1, space="PSUM") as pbp,
                tc.tile_pool(name="pst", bufs=1, space="PSUM") as pstp,
                tc.tile_pool(name="th", bufs=3) as thp,
            ):
                pb = pbp.tile([128, N], F32)
                for kc in range(2):
                    for lo, wd in ((0, 512), (512, 256)):
                        nc.tensor.matmul(
                            pb[:, lo : lo + wd],
                            w1(kc, 1),
                            L2[:, kc, lo : lo + wd],
                            start=(kc == 0),
                            stop=(kc == 1),
                            skip_group_check=True,
                        )
                stt = [
                    pstp.tile([128, NH], F32, tag=f"st{jc}", name=f"st{jc}")
                    for jc in range(6)
                ]
                w2col = cp[:, OFF_W2 : OFF_W2 + 1]
                for i in range(NH):
                    th = thp.tile([128, N], F32)
                    nc.scalar.activation(
                        th[:, :], pb[:, :], AF.Tanh, bias=ab[:, i : i + 1], scale=1.0
                    )
                    for jc in range(6):
                        nc.tensor.matmul(
                            stt[jc][:, i : i + 1],
                            th[:, jc * 128 : (jc + 1) * 128],
                            w2col,
                            start=True,
                            stop=True,
                            skip_group_check=True,
                        )
                st_sb = const.tile([128, 6, NH], F32)
                for jc in range(6):
                    nc.vector.tensor_copy(st_sb[:, jc, :], stt[jc][:, :])
                nc.sync.dma_start(
                    out=st_out[:, :, :].rearrange("c p h -> p c h"),
                    in_=st_sb[:, :, :],
                )

    nc.compile()
    return nc


def _prep(inputs):
    """Host-side packing of all constants (layout prep + algebraic folds)."""
    inp = {k: np.asarray(v) for k, v in inputs.items()}
    x = np.concatenate(
        [inp["sentence_embedded"], inp["root_vec"].reshape(1, -1)], 0
    ).astype(np.float32)                      # (769, 256)
    xT = np.ascontiguousarray(x.T)

    perm = np.concatenate(
        [np.arange(0, H), np.arange(H, 2 * H), np.arange(3 * H, 4 * H),
         np.arange(2 * H, 3 * H)]
    )
    gsc = np.concatenate([np.full(3 * H, 0.5), np.ones(H)]).astype(np.float32)

    cpk = np.zeros((128, CPK), np.float32)
    cpk[:, OFF_XT : OFF_XT + T] = xT[0:128]
    cpk[:, OFF_XT + T : OFF_XT + 2 * T] = xT[128:256]

    brw = np.zeros((1, 4 * 512 + T), np.float32)
    brw[0, 2048 : 2048 + T] = 1.0

    for l in range(NL):
        for d in range(2):
            p = 2 * l + d
            wih = inp["Wih"][l, d][perm] * gsc[:, None]     # (512, in)
            whh_ = inp["Whh"][l, d][perm] * gsc[:, None]    # (512, 128)
            b = (inp["bih"][l, d] + inp["bhh"][l, d])[perm] * gsc
            whh_ = whh_ * 0.5                               # h2 = 2h fold
            if l > 0:
                wih = wih * 0.5
            wihT = wih.T.astype(np.float32)                 # (in=256, 512)
            for kc in range(2):
                o = OFF_WIH + (p * 2 + kc) * 512
                cpk[:, o : o + 512] = wihT[kc * 128 : (kc + 1) * 128]
            o = OFF_WHH + p * 512
            cpk[:, o : o + 512] = whh_.T.astype(np.float32)
            brw[0, p * 512 : (p + 1) * 512] = b

    W1 = inp["W1"].astype(np.float32)
    w1aT = (W1[:, :256] * 0.5).T                            # (256, 128)
    w1bT = (W1[:, 256:] * 0.5).T
    for kc in range(2):
        o = OFF_W1 + kc * 256
        cpk[:, o : o + 128] = w1aT[kc * 128 : (kc + 1) * 128]
        cpk[:, o + 128 : o + 256] = w1bT[kc * 128 : (kc + 1) * 128]
    cpk[:, OFF_B1] = inp["b1"].astype(np.float32)
    cpk[:, OFF_W2] = inp["w2"].astype(np.float32)

    return cpk, brw, float(np.asarray(inp["b2"]).reshape(-1)[0])


def kernel(**inputs) -> np.ndarray:
    cpk, brw, b2 = _prep(inputs)

    if "nc" not in _CACHE:
        _CACHE["nc"] = build_program()
    nc = _CACHE["nc"]

    in_map = {"cpack": cpk, "brow": brw}
    res = run_bass_kernel_spmd(
        nc,
        [dict(in_map) for _ in range(NCORES)],
        core_ids=list(range(NCORES)),
        trace=bool(int(os.environ.get("KERNEL_TRACE", "0"))),
    )
    _CACHE["last_results"] = res

    scores = np.empty((T, N), np.float32)
    for c in range(NCORES):
        st = res.results[c]["st_out"].reshape(N, NH)  # (j, head)
        nh = NH if c == NCORES - 1 else HB
        scores[c * HB : c * HB + nh, :] = st[:, :nh].T
    scores += b2
    mask = np.arange(T)[:, None] == np.arange(N)[None, :]
    scores[mask] = -np.inf
    return scores


# revision 15
# speedup vs baseline: 1.1678x; 1.1678x over previous
"""Trainium2 Bass kernel for nn_DependencyParser (BiLSTM + biaffine-style
pairwise scorer).

Strategy (8 NeuronCores, no collectives):
  - Every core runs the identical 2-layer BiLSTM over the 769-token
    sequence (the recurrence is inherently sequential; replication is
    cheaper than any per-step communication).
  - The pairwise score computation scores[i, j] = w2 . tanh(A_i + B_j + b1)
    is sharded over the head axis i: core c computes heads
    [96*c, 96*c + 97) (97 heads each, one overlapping row; 769 total).
  - Scores are produced transposed (j on partitions, heads on free dim) so
    PSUM evacuation is cheap; the host reassembles/transposes, adds b2 and
    sets the diagonal to -inf.

Cell math (all-tanh formulation; sigma(x) = 0.5*tanh(x/2) + 0.5):
  - Weights for gates i, f, o are pre-scaled by 0.5 host-side, so ONE
    Tanh activation per step covers all 4 gates of both directions
    (fwd + bwd streams share one PSUM gate tensor; strided pair-AP).
  - The kernel tracks h2 = 2h and chat = 2c:
      u     = (Ti + 1) * Tg                    (STT)
      chat  = (Tf * hc + hc) + u, hc = chat/2  (affine_then_add, TS)
      Tc    = tanh(0.5 * chat)                 (ACT, = tanh(c))
      h2    = (To + 1) * Tc                    (STT, = 2h)
    The factors of 2 are folded into Whh / layer-2 Wih / W1 host-side.
  - The recurrent weights, the h2 history, layer-2 input weights, W1/w2
    and the per-head tanh tile run in bf16 (halves PE weight-load time);
    cell state, gate activations and all PSUM accumulation stay fp32.
    Set KERNEL_F32=1 for an all-fp32 build (abs err ~5e-7 vs ~2e-3).
"""

import os
import numpy as np
import ml_dtypes

import concourse.bass as bass
import concourse.mybir as mybir
import concourse.tile as tile
from concourse import bacc
from concourse.bass import ds
from concourse.bass_utils import run_bass_kernel_spmd

F32 = mybir.dt.float32
BF16 = mybir.dt.bfloat16
F16 = mybir.dt.float16
AF = mybir.ActivationFunctionType
ALU = mybir.AluOpType

# weight/state dtype for the recurrent + scores matmul paths.
# f16 keeps PE weight loads at 2 elem/cycle with ~5e-4 end-to-end error;
# f32 is the exact (~5e-7) fallback at ~20% more time.
_DTYPES = {
    "f16": (F16, np.float16),
    "bf16": (BF16, ml_dtypes.bfloat16),
    "f32": (F32, np.float32),
}

T = 769          # sequence length incl. root
N = 768          # dependents
H = 128          # lstm hidden per direction
NL = 2           # layers
NH = 97          # heads per core
HB = 96          # head block stride (base = pid*96)
NCORES = 8

# cpack (f32) layout: x.T | wih layer-1 | b1
OFF_XT = 0
OFF_WIH1 = OFF_XT + 2 * T            # 2 passes x 2 kchunks x 512
OFF_B1 = OFF_WIH1 + 2 * 2 * 512
CPK = OFF_B1 + 1

# cpkh (weight-dtype) layout: wih layer-2 | whh | w1 | w2
OFF_WIH2 = 0                         # 2 passes x 2 kchunks x 512
OFF_WHH = OFF_WIH2 + 2 * 2 * 512
OFF_W1 = OFF_WHH + 4 * 512           # 2 kchunks x 256 ([W1a.T | W1b.T])
OFF_W2 = OFF_W1 + 2 * 256
CPKH = OFF_W2 + 1

_CACHE = {}


def pair(ap_a, ap_b):
    """One AP covering two same-shape regions of the same tensor as an
    extra outer free dim of 2: [a, b]."""
    assert ap_a.tensor is ap_b.tensor
    d = ap_b.offset - ap_a.offset
    assert d > 0, "pair() needs ap_a before ap_b"
    return bass.AP(
        tensor=ap_a.tensor,
        offset=ap_a.offset,
        ap=[list(ap_a.ap[0]), [d, 2]] + [list(x) for x in ap_a.ap[1:]],
    )


def build_program(wdt):
    nc = bacc.Bacc(None, target_bir_lowering=False, num_devices=NCORES)

    cpack = nc.dram_tensor("cpack", (128, CPK), F32, kind="ExternalInput")
    cpkh = nc.dram_tensor("cpkh", (128, CPKH), wdt, kind="ExternalInput")
    # brow: single partition row: 4 pass-bias blocks of 512, then T ones
    brow = nc.dram_tensor("brow", (1, 4 * 512 + T), F32, kind="ExternalInput")
    ascr = nc.dram_tensor("ascr", (128, T), F32, kind="Internal")
    st_out = nc.dram_tensor("st_out", (6, 128, NH), F32, kind="ExternalOutput")

    with tile.TileContext(nc) as tc:
        with (
            tc.tile_pool(name="const", bufs=1) as const,
            tc.tile_pool(name="state", bufs=1) as state,
        ):
            cp = const.tile([128, CPK], F32)
            ch = const.tile([128, CPKH], wdt)
            # split the constant loads across DMA queues for bandwidth
            for a, b in zip(
                [0, T, OFF_WIH1, OFF_WIH1 + 1024, CPK][:-1],
                [0, T, OFF_WIH1, OFF_WIH1 + 1024, CPK][1:],
            ):
                nc.sync.dma_start(out=cp[:, a:b], in_=cpack[:, a:b])
            for a, b in zip(
                [0, 1024, OFF_WHH, OFF_WHH + 1024, OFF_W1, CPKH][:-1],
                [0, 1024, OFF_WHH, OFF_WHH + 1024, OFF_W1, CPKH][1:],
            ):
                nc.sync.dma_start(out=ch[:, a:b], in_=cpkh[:, a:b])
            br = const.tile([1, 4 * 512 + T], F32)
            nc.sync.dma_start(out=br[:, :], in_=brow[:, :])

            def xt(kc):
                return cp[:, OFF_XT + kc * T : OFF_XT + (kc + 1) * T]

            def wih(p, kc):  # (128, 512) lhsT slab for pass p, K-chunk kc
                if p < 2:
                    o = OFF_WIH1 + (p * 2 + kc) * 512
                    return cp[:, o : o + 512]
                o = OFF_WIH2 + ((p - 2) * 2 + kc) * 512
                return ch[:, o : o + 512]

            def whh(p, g):  # (128, 128) lhsT for pass p, gate g
                o = OFF_WHH + p * 512 + g * 128
                return ch[:, o : o + 128]

            def bias_row(p):  # (1, 512)
                return br[0:1, p * 512 : (p + 1) * 512]

            ones_row = br[0:1, 2048 : 2048 + T]

            # persistent LSTM state/scratch tiles
            Hl = [
                state.tile([128, 2, T], wdt, tag=f"H{l}", name=f"H{l}")
                for l in range(NL)
            ]
            Tt = state.tile([128, 2, 4], F32)   # tanh gates [i,f,o,g] per stream
            u_t = state.tile([128, 2], F32)
            ct = state.tile([128, 2], F32)      # chat
            hc = state.tile([128, 2], F32)      # chat/2
            Tc = state.tile([128, 2], F32)      # tanh(c)

            for layer in range(NL):
                nc.vector.memset(hc[:, :], 0.0)  # c0 = 0 per layer
                pf, pb = 2 * layer, 2 * layer + 1
                if layer == 0:
                    X = [xt(0), xt(1)]
                else:
                    X = [Hl[layer - 1][:, 0, :], Hl[layer - 1][:, 1, :]]
                Hcur = Hl[layer]

                # G holds both streams' gate pre-activations for one chunk:
                # fwd chunks cover t in [0,512) then [512,769); bwd chunk A
                # covers t in [257,769) (walked right-to-left), B = [0,257).
                with tc.tile_pool(name=f"ps{layer}", bufs=1, space="PSUM") as psp:
                    G = None
                    fwd_cb = bwd_lo = None

                    def proj(s, p, lo, width):
                        for g in range(4):
                            for kc in range(2):
                                nc.tensor.matmul(
                                    G[:, s, g, 0:width],
                                    wih(p, kc)[:, g * 128 : (g + 1) * 128],
                                    X[kc][:, lo : lo + width],
                                    start=(kc == 0),
                                    stop=False,
                                    skip_group_check=True,
                                )
                            nc.tensor.matmul(
                                G[:, s, g, 0:width],
                                bias_row(p)[:, g * 128 : (g + 1) * 128],
                                ones_row[:, lo : lo + width],
                                start=False,
                                stop=True,
                                skip_group_check=True,
                            )

                    for k in range(T):
                        if k == 0 or k == 512:
                            G = psp.tile([128, 2, 4, 512], F32, tag="G", name="G")
                            if k == 0:
                                fwd_cb, fwd_w = 0, 512
                                bwd_lo, bwd_w = 257, 512
                            else:
                                fwd_cb, fwd_w = 512, 257
                                bwd_lo, bwd_w = 0, 257
                            proj(0, pf, fwd_cb, fwd_w)
                            proj(1, pb, bwd_lo, bwd_w)

                        tf = k            # fwd step position
                        tb = T - 1 - k    # bwd step position
                        cf = tf - fwd_cb
                        cb = tb - bwd_lo

                        if k > 0:
                            for g in range(4):
                                nc.tensor.matmul(
                                    G[:, 0, g, cf : cf + 1],
                                    whh(pf, g),
                                    Hcur[:, 0, tf - 1 : tf],
                                    start=False,
                                    stop=True,
                                    skip_group_check=True,
                                )
                            for g in range(4):
                                nc.tensor.matmul(
                                    G[:, 1, g, cb : cb + 1],
                                    whh(pb, g),
                                    Hcur[:, 1, tb + 1 : tb + 2],
                                    start=False,
                                    stop=True,
                                    skip_group_check=True,
                                )

                        # gates -> tanh, both streams in ONE ACT (PSUM src)
                        gpair = pair(G[:, 0, :, cf], G[:, 1, :, cb])
                        nc.scalar.activation(Tt[:, :, :], gpair, AF.Tanh)

                        # u = (Ti + 1) * Tg        (both streams at once)
                        nc.vector.scalar_tensor_tensor(
                            u_t[:, :], Tt[:, :, 0], 1.0, Tt[:, :, 3],
                            op0=ALU.add, op1=ALU.mult,
                        )
                        # chat = (Tf * hc + hc) + u   (per stream: hc is [P,1])
                        for s in range(2):
                            nc.vector.affine_then_add(
                                ct[:, s : s + 1], Tt[:, s, 1:2], u_t[:, s : s + 1],
                                hc[:, s : s + 1], hc[:, s : s + 1],
                            )
                        nc.vector.tensor_scalar(hc[:, :], ct[:, :], 0.5, None, op0=ALU.mult)
                        # Tc = tanh(chat/2) = tanh(c)
                        nc.scalar.activation(Tc[:, :], ct[:, :], AF.Tanh, scale=0.5)
                        # h2 = (To + 1) * Tc  -> H columns (both streams, pair AP)
                        hpair = pair(Hcur[:, 0, tf : tf + 1], Hcur[:, 1, tb : tb + 1])
                        nc.vector.scalar_tensor_tensor(
                            hpair, Tt[:, :, 2], 1.0, Tc[:, :],
                            op0=ALU.add, op1=ALU.mult,
                        )

            if os.environ.get("KERNEL_DEBUG_DUMP"):
                for l in range(NL):
                    dbg = nc.dram_tensor(f"dbg_h{l}", (128, 2, T), wdt, kind="ExternalOutput")
                    nc.sync.dma_start(out=dbg[:, :, :], in_=Hl[l][:, :, :])

            # ---------------- scores phase ----------------
            L2 = Hl[NL - 1]

            def w1(kc, half):  # (128, 128) lhsT: half 0 = W1a, 1 = W1b
                o = OFF_W1 + kc * 256 + half * 128
                return ch[:, o : o + 128]

            with tc.tile_pool(name="pa", bufs=1, space="PSUM") as pap:
                pa = pap.tile([128, T], F32)
                for kc in range(2):
                    for lo, wd in ((0, 512), (512, 257)):
                        nc.tensor.matmul(
                            pa[:, lo : lo + wd],
                            w1(kc, 0),
                            L2[:, kc, lo : lo + wd],
                            start=(kc == 0),
                            stop=(kc == 1),
                            skip_group_check=True,
                        )
                astar = const.tile([128, T], F32)
                nc.scalar.activation(
                    astar[:, :], pa[:, :], AF.Identity,
                    bias=cp[:, OFF_B1 : OFF_B1 + 1], scale=1.0,
                )
            # head-block gather via DRAM roundtrip (dynamic offset DMA)
            nc.sync.dma_start(out=ascr[:, :], in_=astar[:, :])
            ab = const.tile([128, NH], F32)
            base96 = nc.sync.partition_id() * HB
            nc.sync.dma_start(out=ab[:, :], in_=ascr[:, ds(base96, NH)])

            with (
                tc.tile_pool(name="pb", bufs=1, space="PSUM") as pbp,
                tc.tile_pool(name="pst", bufs=1, space="PSUM") as pstp,
                tc.tile_pool(name="th", bufs=3) as thp,
            ):
                pb = pbp.tile([128, N], F32)
                for kc in range(2):
                    for lo, wd in ((0, 512), (512, 256)):
                        nc.tensor.matmul(
                            pb[:, lo : lo + wd],
                            w1(kc, 1),
                            L2[:, kc, lo : lo + wd],
                            start=(kc == 0),
                            stop=(kc == 1),
                            skip_group_check=True,
                        )
                stt = [
                    pstp.tile([128, NH], F32, tag=f"st{jc}", name=f"st{jc}")
                    for jc in range(6)
                ]
                w2col = ch[:, OFF_W2 : OFF_W2 + 1]
                for i in range(NH):
                    th = thp.tile([128, N], wdt)
                    nc.scalar.activation(
                        th[:, :], pb[:, :], AF.Tanh, bias=ab[:, i : i + 1], scale=1.0
                    )
                    for jc in range(6):
                        nc.tensor.matmul(
                            stt[jc][:, i : i + 1],
                            th[:, jc * 128 : (jc + 1) * 128],
                            w2col,
                            start=True,
                            stop=True,
                            skip_group_check=True,
                        )
                st_sb = const.tile([128, 6, NH], F32)
                for jc in range(6):
                    nc.vector.tensor_copy(st_sb[:, jc, :], stt[jc][:, :])
                nc.sync.dma_start(
                    out=st_out[:, :, :].rearrange("c p h -> p c h"),
                    in_=st_sb[:, :, :],
                )

    nc.compile()
    return nc


def _prep(inputs, np_wdt):
    """Host-side packing of all constants (layout prep + algebraic folds)."""
    inp = {k: np.asarray(v) for k, v in inputs.items()}
    x = np.concatenate(
        [inp["sentence_embedded"], inp["root_vec"].reshape(1, -1)], 0
    ).astype(np.float32)                      # (769, 256)
    xT = np.ascontiguousarray(x.T)

    perm = np.concatenate(
        [np.arange(0, H), np.arange(H, 2 * H), np.arange(3 * H, 4 * H),
         np.arange(2 * H, 3 * H)]
    )
    gsc = np.concatenate([np.full(3 * H, 0.5), np.ones(H)]).astype(np.float32)

    cpk = np.zeros((128, CPK), np.float32)
    cpk[:, OFF_XT : OFF_XT + T] = xT[0:128]
    cpk[:, OFF_XT + T : OFF_XT + 2 * T] = xT[128:256]
    cpk[:, OFF_B1] = inp["b1"].astype(np.float32)

    cph = np.zeros((128, CPKH), np.float32)

    brw = np.zeros((1, 4 * 512 + T), np.float32)
    brw[0, 2048 : 2048 + T] = 1.0

    for l in range(NL):
        for d in range(2):
            p = 2 * l + d
            wihm = inp["Wih"][l, d][perm] * gsc[:, None]    # (512, in)
            whhm = inp["Whh"][l, d][perm] * gsc[:, None]    # (512, 128)
            b = (inp["bih"][l, d] + inp["bhh"][l, d])[perm] * gsc
            whhm = whhm * 0.5                               # h2 = 2h fold
            if l > 0:
                wihm = wihm * 0.5
            wihT = wihm.T.astype(np.float32)                # (in=256, 512)
            for kc in range(2):
                blk = wihT[kc * 128 : (kc + 1) * 128]
                if l == 0:
                    o = OFF_WIH1 + (p * 2 + kc) * 512
                    cpk[:, o : o + 512] = blk
                else:
                    o = OFF_WIH2 + ((p - 2) * 2 + kc) * 512
                    cph[:, o : o + 512] = blk
            o = OFF_WHH + p * 512
            cph[:, o : o + 512] = whhm.T.astype(np.float32)
            brw[0, p * 512 : (p + 1) * 512] = b

    W1 = inp["W1"].astype(np.float32)
    w1aT = (W1[:, :256] * 0.5).T                            # (256, 128)
    w1bT = (W1[:, 256:] * 0.5).T
    for kc in range(2):
        o = OFF_W1 + kc * 256
        cph[:, o : o + 128] = w1aT[kc * 128 : (kc + 1) * 128]
        cph[:, o + 128 : o + 256] = w1bT[kc * 128 : (kc + 1) * 128]
    cph[:, OFF_W2] = inp["w2"].astype(np.float32)

    return cpk, cph.astype(np_wdt), brw, float(np.asarray(inp["b2"]).reshape(-1)[0])


def kernel(**inputs) -> np.ndarray:
    dname = os.environ.get("KERNEL_DTYPE", "f16")
    wdt, np_wdt = _DTYPES[dname]

    cpk, cph, brw, b2 = _prep(inputs, np_wdt)

    key = ("nc", dname)
    if key not in _CACHE:
        _CACHE[key] = build_program(wdt)
    nc = _CACHE[key]

    in_map = {"cpack": cpk, "cpkh": cph, "brow": brw}
    res = run_bass_kernel_spmd(
        nc,
        [dict(in_map) for _ in range(NCORES)],
        core_ids=list(range(NCORES)),
    )
    _CACHE["last_results"] = res

    scores = np.empty((T, N), np.float32)
    for c in range(NCORES):
        st = res.results[c]["st_out"].reshape(N, NH)  # (j, head)
        nh = NH if c == NCORES - 1 else HB
        scores[c * HB : c * HB + nh, :] = st[:, :nh].T
    scores += b2
    mask = np.arange(T)[:, None] == np.arange(N)[None, :]
    scores[mask] = -np.inf
    return scores
